# revision 15
# baseline (speedup 1.0000x reference)
"""Trainium2 Bass kernel for nn_ExpertAttentionHead (attention + SwiGLU MLP).

Sharding (8 cores): DP over batch (2 groups of 4 cores) x TP within group.
  - Attention: heads split 4-way (4 heads/core). QKV projections computed in
    transposed layout (hd on partitions) from host-pretransposed x^T.
  - Per-head AllGather of the attention output (transposed layout) within
    each group, overlapped with attention of the remaining heads.
  - MLP: W1/Vg column-sharded (E/4 per core), W2 row-sharded; the fp32
    partial outputs are ReduceScattered in 4 column chunks overlapped with
    compute; the host concatenates row slices.

Everything is hardcoded for B=2, L=2048, D=2048, H=16, HD=128, E=8192.
"""

import sys

import numpy as np

sys.path.insert(0, "/opt/trn_rl_repo")

import ml_dtypes

BF16 = ml_dtypes.bfloat16

B, L, D = 2, 2048, 2048
H, HD = 16, 128
E = 8192
SCALE = float(np.sqrt(HD))

P = 128
NCORES = 8
TP = 4  # tensor-parallel ranks per group
NH = H // TP  # local heads = 4
HSL = NH * HD  # head col slice width = 512
EL = E // TP  # local E = 2048
LT = L // P  # 16 query tiles
DC = D // P  # 16 contraction chunks
ROWS = L // TP  # 512 output rows per core
RS_CHUNKS = [512, 512, 512, 256, 256]  # ReduceScatter column chunks
RS_OFFS = [0, 512, 1024, 1536, 1792]
ND = len(RS_CHUNKS)

_PROGRAM = None


def _build_program(debug_outputs=False, no_cc=False, dma_transpose=False):
    import concourse.bacc as bacc
    import concourse.mybir as mybir
    import concourse.tile as tile
    from concourse.masks import make_identity

    fp32 = mybir.dt.float32
    bf16 = mybir.dt.bfloat16

    nc = bacc.Bacc("TRN2", target_bir_lowering=False, debug=False,
                   num_devices=NCORES)

    # ---- I/O ----
    xT = nc.dram_tensor("xT", [D, L], bf16, kind="ExternalInput")
    wq = nc.dram_tensor("wq", [D, HSL], bf16, kind="ExternalInput")
    wk = nc.dram_tensor("wk", [D, HSL], bf16, kind="ExternalInput")
    wv = nc.dram_tensor("wv", [D, HSL], bf16, kind="ExternalInput")
    # host-tiled: (E-tile, p=d_in_chunk, d-chunk, e_cols)
    w1t = nc.dram_tensor("w1t", [EL // P, P, DC, P], bf16, kind="ExternalInput")
    vgt = nc.dram_tensor("vgt", [EL // P, P, DC, P], bf16, kind="ExternalInput")
    w2 = nc.dram_tensor("w2", [EL, D], bf16, kind="ExternalInput")
    tri = nc.dram_tensor("tri", [P, P], bf16, kind="ExternalInput")

    y_out = nc.dram_tensor("y", [ROWS, D], fp32, kind="ExternalOutput")

    # ---- collective bounce buffers (internal DRAM) ----
    ag_in_h = [nc.dram_tensor(f"ag_in_{h}", [P, L], bf16) for h in range(NH)]
    ag_out_h = [nc.dram_tensor(f"ag_out_{h}", [TP * P, L], bf16)
                for h in range(NH)]
    rs_in_n = [nc.dram_tensor(f"rs_in_{n}", [L, RS_CHUNKS[n]], fp32)
               for n in range(ND)]
    rs_out_n = [nc.dram_tensor(f"rs_out_{n}", [ROWS, RS_CHUNKS[n]], fp32)
                for n in range(ND)]

    groups = [[0, 1, 2, 3], [4, 5, 6, 7]]

    dbg = {}
    if debug_outputs:
        dbg["qT"] = nc.dram_tensor("dbg_qT", [NH, P, L], fp32, kind="ExternalOutput")
        dbg["kT"] = nc.dram_tensor("dbg_kT", [NH, P, L], fp32, kind="ExternalOutput")
        dbg["v"] = nc.dram_tensor("dbg_v", [LT, P, HSL], fp32, kind="ExternalOutput")
        dbg["outT"] = nc.dram_tensor("dbg_outT", [NH, P, L], fp32,
                                     kind="ExternalOutput")

    with tile.TileContext(nc) as tc, \
         tc.tile_pool(name="consts", bufs=1) as consts:
        identity = consts.tile([P, P], bf16)
        make_identity(nc, identity[:])
        tri_sb = consts.tile([P, P], bf16)
        nc.sync.dma_start(tri_sb[:], tri[:])

        # persistent across stage 1+2
        with tc.tile_pool(name="attn_persist", bufs=1) as persist:
            qT_sb = persist.tile([P, NH, L], bf16)
            kT_sb = persist.tile([P, NH, L], bf16)
            v_sb = persist.tile([P, LT, HSL], bf16)
            kbar_sb = persist.tile([P, NH], bf16)
            outT_sb = persist.tile([P, NH, L], bf16)

            # ---------------- stage 1: QKV projections ----------------
            with tc.tile_pool(name="proj", bufs=1) as proj, \
                 tc.tile_pool(name="proj_ps", bufs=2, space="PSUM") as proj_ps:
                wq_sb = proj.tile([P, DC, HSL], bf16, tag="wq")
                wq_r = wq.rearrange("(c p) h -> p c h", p=P)
                for h in range(NH):
                    nc.sync.dma_start(wq_sb[:, :, h * P:(h + 1) * P],
                                      wq_r[:, :, h * P:(h + 1) * P])
                xT_sb = proj.tile([P, DC, L], bf16, tag="xT")
                xT_r = xT.rearrange("(c p) l -> p c l", p=P)
                for i in range(DC):
                    nc.sync.dma_start(xT_sb[:, i:i + 1, :],
                                      xT_r[:, i:i + 1, :])
                wk_sb = proj.tile([P, DC, HSL], bf16, tag="wk")
                wk_r = wk.rearrange("(c p) h -> p c h", p=P)
                for h in range(NH):
                    nc.sync.dma_start(wk_sb[:, :, h * P:(h + 1) * P],
                                      wk_r[:, :, h * P:(h + 1) * P])
                wv_sb = proj.tile([P, DC, HSL], bf16, tag="wv")
                nc.sync.dma_start(wv_sb[:], wv.rearrange("(c p) h -> p c h", p=P))

                # q^T, k^T per head: (hd=128, pos) = sum_d W[:,h]^T x^T
                for w_sb, dst in ((wq_sb, qT_sb), (wk_sb, kT_sb)):
                    for h in range(NH):
                        for pc in range(L // 512):
                            ps = proj_ps.tile([P, 512], fp32, tag="proj_ps")
                            for dc in range(DC):
                                nc.tensor.matmul(
                                    ps[:],
                                    lhsT=w_sb[:, dc, h * P:(h + 1) * P],
                                    rhs=xT_sb[:, dc, pc * 512:(pc + 1) * 512],
                                    start=(dc == 0), stop=(dc == DC - 1),
                                )
                            nc.scalar.copy(dst[:, h, pc * 512:(pc + 1) * 512], ps[:])
                # v in normal layout: (pos, hd-cols)
                for pt in range(LT):
                    ps = proj_ps.tile([P, HSL], fp32, tag="proj_ps")
                    for dc in range(DC):
                        nc.tensor.matmul(
                            ps[:],
                            lhsT=xT_sb[:, dc, pt * P:(pt + 1) * P],
                            rhs=wv_sb[:, dc, :],
                            start=(dc == 0), stop=(dc == DC - 1),
                        )
                    nc.scalar.copy(v_sb[:, pt, :], ps[:])

            # k_bar per head (sum over keys) for the Reynolds row-mean
            for h in range(NH):
                kbar_f = persist.tile([P, 1], fp32, tag="kbar_f")
                nc.vector.reduce_sum(kbar_f[:], kT_sb[:, h, :],
                                     axis=mybir.AxisListType.X)
                nc.vector.tensor_copy(kbar_sb[:, h:h + 1], kbar_f[:])

            if debug_outputs:
                for h in range(NH):
                    st = persist.tile([P, L], fp32, tag="dbg_cast")
                    nc.vector.tensor_copy(st[:], qT_sb[:, h, :])
                    nc.sync.dma_start(dbg["qT"][h], st[:])
                for h in range(NH):
                    st = persist.tile([P, L], fp32, tag="dbg_cast")
                    nc.vector.tensor_copy(st[:], kT_sb[:, h, :])
                    nc.sync.dma_start(dbg["kT"][h], st[:])
                for pt in range(LT):
                    st = persist.tile([P, HSL], fp32, tag="dbg_cast2")
                    nc.vector.tensor_copy(st[:], v_sb[:, pt, :])
                    nc.sync.dma_start(dbg["v"][pt], st[:])

            # ---------------- stage 2: attention + per-head AllGather ------
            with tc.tile_pool(name="attn", bufs=3) as attn, \
                 tc.tile_pool(name="ps_s", bufs=2, space="PSUM") as ps_s_pool, \
                 tc.tile_pool(name="ps_r", bufs=1, space="PSUM") as ps_r_pool, \
                 tc.tile_pool(name="ps_t", bufs=2, space="PSUM") as ps_t_pool, \
                 tc.tile_pool(name="ps_o", bufs=2, space="PSUM") as ps_o_pool:
                for h in range(NH):
                    for qt in range(LT):
                        ncb = qt + 1          # causal key blocks
                        cw = ncb * P          # causal width
                        qsl = slice(qt * P, (qt + 1) * P)

                        # Reynolds row-mean via k_bar: rowsum = q . k_bar
                        ps_row = ps_r_pool.tile([P, 1], fp32, tag="ps_row")
                        nc.tensor.matmul(ps_row[:], lhsT=qT_sb[:, h, qsl],
                                         rhs=kbar_sb[:, h:h + 1],
                                         start=True, stop=True)
                        bias_t = attn.tile([P, 1], fp32, tag="bias")
                        nc.vector.tensor_scalar_mul(
                            bias_t[:], ps_row[:], 0.5 / (SCALE * L))

                        e_t = attn.tile([P, L], bf16, tag="e")
                        dpart = attn.tile([P, 8], fp32, tag="dpart")
                        npart = 0
                        # causal chunks of <=512 keys
                        for c0 in range(0, cw, 512):
                            w = min(512, cw - c0)
                            ps_sc = ps_s_pool.tile([P, 512], fp32, tag="ps_s")
                            nc.tensor.matmul(
                                ps_sc[:, :w], lhsT=qT_sb[:, h, qsl],
                                rhs=kT_sb[:, h, c0:c0 + w],
                                start=True, stop=True)
                            # exp(0.5*s/SCALE + bias), accumulate row-sums
                            pre_w = w if c0 + w <= qt * P else w - P
                            if pre_w > 0:
                                nc.scalar.activation(
                                    e_t[:, c0:c0 + pre_w], ps_sc[:, :pre_w],
                                    mybir.ActivationFunctionType.Exp,
                                    bias=bias_t[:], scale=0.5 / SCALE,
                                    accum_out=dpart[:, npart:npart + 1])
                                npart += 1
                            if c0 + w > qt * P:
                                # diagonal block: exp, tri-mask, row-sum
                                doff = qt * P - c0
                                nc.scalar.activation(
                                    e_t[:, qt * P:qt * P + P],
                                    ps_sc[:, doff:doff + P],
                                    mybir.ActivationFunctionType.Exp,
                                    bias=bias_t[:], scale=0.5 / SCALE)
                                nc.vector.tensor_tensor(
                                    e_t[:, qt * P:qt * P + P],
                                    e_t[:, qt * P:qt * P + P], tri_sb[:],
                                    mybir.AluOpType.mult)
                                nc.vector.reduce_sum(
                                    dpart[:, npart:npart + 1],
                                    e_t[:, qt * P:qt * P + P],
                                    axis=mybir.AxisListType.X)
                                npart += 1

                        denom = attn.tile([P, 1], fp32, tag="denom")
                        nc.vector.reduce_sum(denom[:], dpart[:, :npart],
                                             axis=mybir.AxisListType.X)
                        recip = attn.tile([P, 1], fp32, tag="recip")
                        nc.vector.reciprocal(recip[:], denom[:])
                        nc.vector.tensor_scalar_mul(e_t[:, :cw], e_t[:, :cw],
                                                    recip[:])

                        # transpose attn blocks, then attn @ v
                        aT = attn.tile([P, L], bf16, tag="aT")
                        for kt in range(ncb):
                            if dma_transpose:
                                nc.sync.dma_start_transpose(
                                    aT[:, kt * P:(kt + 1) * P],
                                    e_t[:, kt * P:(kt + 1) * P])
                                continue
                            ps_t = ps_t_pool.tile([P, P], bf16, tag="ps_t")
                            nc.tensor.transpose(
                                ps_t[:], e_t[:, kt * P:(kt + 1) * P],
                                identity[:])
                            nc.vector.tensor_copy(aT[:, kt * P:(kt + 1) * P],
                                                  ps_t[:])
                        ps_o = ps_o_pool.tile([P, P], fp32, tag="ps_o")
                        for kt in range(ncb):
                            nc.tensor.matmul(
                                ps_o[:], lhsT=v_sb[:, kt, h * P:(h + 1) * P],
                                rhs=aT[:, kt * P:(kt + 1) * P],
                                start=(kt == 0), stop=(kt == ncb - 1))
                        nc.scalar.copy(outT_sb[:, h, qsl], ps_o[:])

                    # head h complete: AllGather its outT slice, then load
                    # the gathered rank blocks into oT (overlaps next heads)
                    nc.sync.dma_start(ag_in_h[h][:], outT_sb[:, h, :])
                    if no_cc:
                        nc.sync.dma_start(ag_out_h[h][:P, :], ag_in_h[h][:])
                    else:
                        nc.gpsimd.collective_compute(
                            "AllGather", mybir.AluOpType.bypass,
                            replica_groups=groups,
                            ins=[ag_in_h[h][:]], outs=[ag_out_h[h][:]])

            if debug_outputs:
                for h in range(NH):
                    st = persist.tile([P, L], fp32, tag="dbg_cast")
                    nc.vector.tensor_copy(st[:], outT_sb[:, h, :])
                    nc.sync.dma_start(dbg["outT"][h], st[:])

        # ---------------- stage 4: MLP ----------------
        with tc.tile_pool(name="mlp_persist", bufs=1) as mlpp, \
             tc.tile_pool(name="mlp_y", bufs=2) as mlp_y:
            hT_sb = mlpp.tile([P, EL // P, L], bf16)

            # phase A: hT = silu(oT.T W1).T * (oT.T Vg).T, column-sharded
            with tc.tile_pool(name="mlp_h", bufs=1) as mlp_h, \
                 tc.tile_pool(name="mlp_w", bufs=2) as mlp_w, \
                 tc.tile_pool(name="mlp_ps", bufs=2, space="PSUM") as mlp_ps, \
                 tc.tile_pool(name="mlp_tmp", bufs=2) as mlp_tmp:
                oT_sb = mlp_h.tile([P, DC, L], bf16)
                for h in range(NH):
                    for rr in range(TP):
                        nc.sync.dma_start(
                            oT_sb[:, rr * NH + h, :],
                            ag_out_h[h][rr * P:(rr + 1) * P, :])
                for et in range(EL // P):
                    w1_sb = mlp_w.tile([P, DC, P], bf16, tag="w1")
                    nc.sync.dma_start(w1_sb[:], w1t[et])
                    vg_sb = mlp_w.tile([P, DC, P], bf16, tag="vg")
                    nc.sync.dma_start(vg_sb[:], vgt[et])
                    for pc in range(L // 512):
                        psl = slice(pc * 512, (pc + 1) * 512)
                        ps_h1 = mlp_ps.tile([P, 512], fp32, tag="ps_h1")
                        for dc in range(DC):
                            nc.tensor.matmul(
                                ps_h1[:], lhsT=w1_sb[:, dc, :],
                                rhs=oT_sb[:, dc, psl],
                                start=(dc == 0), stop=(dc == DC - 1))
                        ps_g1 = mlp_ps.tile([P, 512], fp32, tag="ps_g1")
                        for dc in range(DC):
                            nc.tensor.matmul(
                                ps_g1[:], lhsT=vg_sb[:, dc, :],
                                rhs=oT_sb[:, dc, psl],
                                start=(dc == 0), stop=(dc == DC - 1))
                        sil = mlp_tmp.tile([P, 512], bf16, tag="sil")
                        nc.scalar.activation(sil[:], ps_h1[:],
                                             mybir.ActivationFunctionType.Silu)
                        nc.vector.tensor_tensor(hT_sb[:, et, psl], sil[:],
                                                ps_g1[:], mybir.AluOpType.mult)

            # phase B: y = h @ W2 (row-sharded W2) -> fp32 partials,
            # ReduceScattered per 512-column chunk (overlaps compute)
            with tc.tile_pool(name="y_ps", bufs=3, space="PSUM") as y_ps, \
                 tc.tile_pool(name="y_tmp", bufs=3) as y_tmp:
                for nd in range(ND):
                    w0, cw2 = RS_OFFS[nd], RS_CHUNKS[nd]
                    w2_sb = mlp_y.tile([P, EL // P, 512], bf16, tag="w2")
                    nc.sync.dma_start(
                        w2_sb[:, :, :cw2],
                        w2[:, w0:w0 + cw2].rearrange("(c p) d -> p c d", p=P))
                    for pt in range(LT):
                        ps_y = y_ps.tile([P, 512], fp32, tag="ps_y")
                        for et in range(EL // P):
                            nc.tensor.matmul(
                                ps_y[:, :cw2],
                                lhsT=hT_sb[:, et, pt * P:(pt + 1) * P],
                                rhs=w2_sb[:, et, :cw2],
                                start=(et == 0), stop=(et == EL // P - 1))
                        yst = y_tmp.tile([P, 512], fp32, tag="yst")
                        nc.scalar.copy(yst[:, :cw2], ps_y[:, :cw2])
                        nc.sync.dma_start(
                            rs_in_n[nd][pt * P:(pt + 1) * P, :], yst[:, :cw2])
                    if no_cc:
                        nc.sync.dma_start(rs_out_n[nd][:], rs_in_n[nd][:ROWS, :])
                    else:
                        nc.gpsimd.collective_compute(
                            "ReduceScatter", mybir.AluOpType.add,
                            replica_groups=groups,
                            ins=[rs_in_n[nd][:]], outs=[rs_out_n[nd][:]])
                    nc.sync.dma_start(y_out[:, w0:w0 + cw2], rs_out_n[nd][:])

    nc.compile()
    return nc


def _prep_inputs(x, Wq, Wk, Wv, W1, Vg, W2):
    """Build the 8 per-core input maps (host-side shard + cast + tile)."""
    tri = np.tril(np.ones((P, P), np.float32)).astype(BF16)
    in_maps = []
    for core in range(NCORES):
        g, r = divmod(core, TP)
        xT = np.ascontiguousarray(x[g].T).astype(BF16)
        hsl = slice(r * HSL, (r + 1) * HSL)
        esl = slice(r * EL, (r + 1) * EL)
        w1 = W1[:, esl].astype(BF16)  # (D, EL)
        # tile to (EL//P, P, DC, P): (et, p, dc, e) = w1[dc*P+p, et*P+e]
        w1t = np.ascontiguousarray(
            w1.reshape(DC, P, EL // P, P).transpose(2, 1, 0, 3))
        vg = Vg[:, esl].astype(BF16)
        vgt = np.ascontiguousarray(
            vg.reshape(DC, P, EL // P, P).transpose(2, 1, 0, 3))
        in_maps.append({
            "xT": xT,
            "wq": np.ascontiguousarray(Wq[:, hsl]).astype(BF16),
            "wk": np.ascontiguousarray(Wk[:, hsl]).astype(BF16),
            "wv": np.ascontiguousarray(Wv[:, hsl]).astype(BF16),
            "w1t": w1t,
            "vgt": vgt,
            "w2": np.ascontiguousarray(W2[esl, :]).astype(BF16),
            "tri": tri,
        })
    return in_maps


def run(inputs, trace=False, debug_outputs=False):
    """Compile (cached) and run; returns (y, BassKernelResults)."""
    global _PROGRAM
    from concourse import bass_utils

    if debug_outputs:
        nc = _build_program(debug_outputs=True)
    else:
        if _PROGRAM is None:
            _PROGRAM = _build_program()
        nc = _PROGRAM

    in_maps = _prep_inputs(inputs["x"], inputs["Wq"], inputs["Wk"],
                           inputs["Wv"], inputs["W1"], inputs["Vg"],
                           inputs["W2"])
    res = bass_utils.run_bass_kernel_spmd(
        nc, in_maps, core_ids=list(range(NCORES)), trace=trace)
    y = np.empty((B, L, D), np.float32)
    for core in range(NCORES):
        g, r = divmod(core, TP)
        y[g, r * ROWS:(r + 1) * ROWS, :] = res.results[core]["y"]
    return y, res


def kernel(**inputs):
    y, _ = run(inputs)
    return y


# revision 19
# speedup vs baseline: 1.0099x; 1.0099x over previous
"""Trainium2 Bass kernel for nn_ExpertAttentionHead (attention + SwiGLU MLP).

Sharding (8 cores): DP over batch (2 groups of 4 cores) x TP within group.
  - Attention: heads split 4-way (4 heads/core). QKV projections computed in
    transposed layout (hd on partitions) from host-pretransposed x^T.
  - Per-head AllGather of the attention output (transposed layout) within
    each group, overlapped with attention of the remaining heads.
  - MLP: W1/Vg column-sharded (E/4 per core), W2 row-sharded; the fp32
    partial outputs are ReduceScattered in 4 column chunks overlapped with
    compute; the host concatenates row slices.

Everything is hardcoded for B=2, L=2048, D=2048, H=16, HD=128, E=8192.
"""

import sys

import numpy as np

sys.path.insert(0, "/opt/trn_rl_repo")

import ml_dtypes

BF16 = ml_dtypes.bfloat16

B, L, D = 2, 2048, 2048
H, HD = 16, 128
E = 8192
SCALE = float(np.sqrt(HD))

P = 128
NCORES = 8
TP = 4  # tensor-parallel ranks per group
NH = H // TP  # local heads = 4
HSL = NH * HD  # head col slice width = 512
EL = E // TP  # local E = 2048
LT = L // P  # 16 query tiles
DC = D // P  # 16 contraction chunks
ROWS = L // TP  # 512 output rows per core
RS_CHUNKS = [512, 512, 512, 256, 128, 128]  # ReduceScatter column chunks
RS_OFFS = [0, 512, 1024, 1536, 1792, 1920]
ND = len(RS_CHUNKS)

_PROGRAM = None


def _build_program(debug_outputs=False, no_cc=False, dma_transpose=False):
    import concourse.bacc as bacc
    import concourse.mybir as mybir
    import concourse.tile as tile
    from concourse.masks import make_identity

    fp32 = mybir.dt.float32
    bf16 = mybir.dt.bfloat16

    nc = bacc.Bacc("TRN2", target_bir_lowering=False, debug=False,
                   num_devices=NCORES)

    # ---- I/O ----
    xT = nc.dram_tensor("xT", [D, L], bf16, kind="ExternalInput")
    wq = nc.dram_tensor("wq", [D, HSL], bf16, kind="ExternalInput")
    wk = nc.dram_tensor("wk", [D, HSL], bf16, kind="ExternalInput")
    wv = nc.dram_tensor("wv", [D, HSL], bf16, kind="ExternalInput")
    # host-tiled: (E-tile, p=d_in_chunk, d-chunk, e_cols)
    w1t = nc.dram_tensor("w1t", [EL // P, P, DC, P], bf16, kind="ExternalInput")
    vgt = nc.dram_tensor("vgt", [EL // P, P, DC, P], bf16, kind="ExternalInput")
    w2 = nc.dram_tensor("w2", [EL, D], bf16, kind="ExternalInput")
    tri = nc.dram_tensor("tri", [P, P], bf16, kind="ExternalInput")

    y_out = nc.dram_tensor("y", [ROWS, D], fp32, kind="ExternalOutput")

    # ---- collective bounce buffers (internal DRAM) ----
    ag_in_h = [nc.dram_tensor(f"ag_in_{h}", [P, L], bf16) for h in range(NH)]
    ag_out_h = [nc.dram_tensor(f"ag_out_{h}", [TP * P, L], bf16)
                for h in range(NH)]
    rs_in_n = [nc.dram_tensor(f"rs_in_{n}", [L, RS_CHUNKS[n]], fp32)
               for n in range(ND)]
    rs_out_n = [nc.dram_tensor(f"rs_out_{n}", [ROWS, RS_CHUNKS[n]], fp32)
                for n in range(ND)]

    groups = [[0, 1, 2, 3], [4, 5, 6, 7]]

    dbg = {}
    if debug_outputs:
        dbg["qT"] = nc.dram_tensor("dbg_qT", [NH, P, L], fp32, kind="ExternalOutput")
        dbg["kT"] = nc.dram_tensor("dbg_kT", [NH, P, L], fp32, kind="ExternalOutput")
        dbg["v"] = nc.dram_tensor("dbg_v", [LT, P, HSL], fp32, kind="ExternalOutput")
        dbg["outT"] = nc.dram_tensor("dbg_outT", [NH, P, L], fp32,
                                     kind="ExternalOutput")

    with tile.TileContext(nc) as tc, \
         tc.tile_pool(name="consts", bufs=1) as consts:
        identity = consts.tile([P, P], bf16)
        make_identity(nc, identity[:])
        tri_sb = consts.tile([P, P], bf16)
        nc.sync.dma_start(tri_sb[:], tri[:])

        # persistent across stage 1+2
        with tc.tile_pool(name="attn_persist", bufs=1) as persist:
            qT_sb = persist.tile([P, NH, L], bf16)
            kT_sb = persist.tile([P, NH, L], bf16)
            v_sb = persist.tile([P, LT, HSL], bf16)
            kbar_sb = persist.tile([P, NH], bf16)
            outT_sb = persist.tile([P, NH, L], bf16)

            # ---------------- stage 1: QKV projections ----------------
            with tc.tile_pool(name="proj", bufs=1) as proj, \
                 tc.tile_pool(name="proj_ps", bufs=2, space="PSUM") as proj_ps:
                wq_sb = proj.tile([P, DC, HSL], bf16, tag="wq")
                wq_r = wq.rearrange("(c p) h -> p c h", p=P)
                xT_sb = proj.tile([P, DC, L], bf16, tag="xT")
                xT_r = xT.rearrange("(c p) l -> p c l", p=P)
                wk_sb = proj.tile([P, DC, HSL], bf16, tag="wk")
                wk_r = wk.rearrange("(c p) h -> p c h", p=P)
                wv_sb = proj.tile([P, DC, HSL], bf16, tag="wv")
                # DMA emission order == consumption order: a tiny first
                # weight chunk, the first pos-quarter of x^T in d-subchunks,
                # then the rest in the order the projection chains need it.
                nc.sync.dma_start(wq_sb[:, 0:1, 0:P], wq_r[:, 0:1, 0:P])
                for s in range(4):
                    nc.sync.dma_start(
                        xT_sb[:, 4 * s:4 * (s + 1), 0:512],
                        xT_r[:, 4 * s:4 * (s + 1), 0:512])
                nc.sync.dma_start(wq_sb[:, 1:, 0:P], wq_r[:, 1:, 0:P])
                for h in range(1, NH):
                    nc.sync.dma_start(wq_sb[:, :, h * P:(h + 1) * P],
                                      wq_r[:, :, h * P:(h + 1) * P])
                for j in range(1, 4):
                    nc.sync.dma_start(xT_sb[:, :, j * 512:(j + 1) * 512],
                                      xT_r[:, :, j * 512:(j + 1) * 512])
                for h in range(NH):
                    nc.sync.dma_start(wk_sb[:, :, h * P:(h + 1) * P],
                                      wk_r[:, :, h * P:(h + 1) * P])
                nc.sync.dma_start(wv_sb[:], wv.rearrange("(c p) h -> p c h", p=P))

                # q^T, k^T per head: (hd=128, pos) = sum_d W[:,h]^T x^T
                # pc-outer so the first chains only need the first x^T quarter
                for w_sb, dst in ((wq_sb, qT_sb), (wk_sb, kT_sb)):
                    for pc in range(L // 512):
                        for h in range(NH):
                            ps = proj_ps.tile([P, 512], fp32, tag="proj_ps")
                            for dc in range(DC):
                                nc.tensor.matmul(
                                    ps[:],
                                    lhsT=w_sb[:, dc, h * P:(h + 1) * P],
                                    rhs=xT_sb[:, dc, pc * 512:(pc + 1) * 512],
                                    start=(dc == 0), stop=(dc == DC - 1),
                                )
                            nc.scalar.copy(dst[:, h, pc * 512:(pc + 1) * 512], ps[:])
                # v in normal layout: (pos, hd-cols)
                for pt in range(LT):
                    ps = proj_ps.tile([P, HSL], fp32, tag="proj_ps")
                    for dc in range(DC):
                        nc.tensor.matmul(
                            ps[:],
                            lhsT=xT_sb[:, dc, pt * P:(pt + 1) * P],
                            rhs=wv_sb[:, dc, :],
                            start=(dc == 0), stop=(dc == DC - 1),
                        )
                    nc.scalar.copy(v_sb[:, pt, :], ps[:])

            # k_bar per head (sum over keys) for the Reynolds row-mean
            for h in range(NH):
                kbar_f = persist.tile([P, 1], fp32, tag="kbar_f")
                nc.vector.reduce_sum(kbar_f[:], kT_sb[:, h, :],
                                     axis=mybir.AxisListType.X)
                nc.vector.tensor_copy(kbar_sb[:, h:h + 1], kbar_f[:])

            if debug_outputs:
                for h in range(NH):
                    st = persist.tile([P, L], fp32, tag="dbg_cast")
                    nc.vector.tensor_copy(st[:], qT_sb[:, h, :])
                    nc.sync.dma_start(dbg["qT"][h], st[:])
                for h in range(NH):
                    st = persist.tile([P, L], fp32, tag="dbg_cast")
                    nc.vector.tensor_copy(st[:], kT_sb[:, h, :])
                    nc.sync.dma_start(dbg["kT"][h], st[:])
                for pt in range(LT):
                    st = persist.tile([P, HSL], fp32, tag="dbg_cast2")
                    nc.vector.tensor_copy(st[:], v_sb[:, pt, :])
                    nc.sync.dma_start(dbg["v"][pt], st[:])

            # ---------------- stage 2: attention + per-head AllGather ------
            with tc.tile_pool(name="attn", bufs=3) as attn, \
                 tc.tile_pool(name="ps_s", bufs=2, space="PSUM") as ps_s_pool, \
                 tc.tile_pool(name="ps_r", bufs=1, space="PSUM") as ps_r_pool, \
                 tc.tile_pool(name="ps_t", bufs=2, space="PSUM") as ps_t_pool, \
                 tc.tile_pool(name="ps_o", bufs=2, space="PSUM") as ps_o_pool:
                for h in range(NH):
                    for qt in range(LT):
                        ncb = qt + 1          # causal key blocks
                        cw = ncb * P          # causal width
                        qsl = slice(qt * P, (qt + 1) * P)

                        # Reynolds row-mean via k_bar: rowsum = q . k_bar
                        ps_row = ps_r_pool.tile([P, 1], fp32, tag="ps_row")
                        nc.tensor.matmul(ps_row[:], lhsT=qT_sb[:, h, qsl],
                                         rhs=kbar_sb[:, h:h + 1],
                                         start=True, stop=True)
                        bias_t = attn.tile([P, 1], fp32, tag="bias")
                        nc.vector.tensor_scalar_mul(
                            bias_t[:], ps_row[:], 0.5 / (SCALE * L))

                        e_t = attn.tile([P, L], bf16, tag="e")
                        dpart = attn.tile([P, 8], fp32, tag="dpart")
                        npart = 0
                        # causal chunks of <=512 keys
                        for c0 in range(0, cw, 512):
                            w = min(512, cw - c0)
                            ps_sc = ps_s_pool.tile([P, 512], fp32, tag="ps_s")
                            nc.tensor.matmul(
                                ps_sc[:, :w], lhsT=qT_sb[:, h, qsl],
                                rhs=kT_sb[:, h, c0:c0 + w],
                                start=True, stop=True)
                            # exp(0.5*s/SCALE + bias), accumulate row-sums
                            pre_w = w if c0 + w <= qt * P else w - P
                            if pre_w > 0:
                                nc.scalar.activation(
                                    e_t[:, c0:c0 + pre_w], ps_sc[:, :pre_w],
                                    mybir.ActivationFunctionType.Exp,
                                    bias=bias_t[:], scale=0.5 / SCALE,
                                    accum_out=dpart[:, npart:npart + 1])
                                npart += 1
                            if c0 + w > qt * P:
                                # diagonal block: exp, tri-mask, row-sum
                                doff = qt * P - c0
                                nc.scalar.activation(
                                    e_t[:, qt * P:qt * P + P],
                                    ps_sc[:, doff:doff + P],
                                    mybir.ActivationFunctionType.Exp,
                                    bias=bias_t[:], scale=0.5 / SCALE)
                                nc.vector.tensor_tensor(
                                    e_t[:, qt * P:qt * P + P],
                                    e_t[:, qt * P:qt * P + P], tri_sb[:],
                                    mybir.AluOpType.mult)
                                nc.vector.reduce_sum(
                                    dpart[:, npart:npart + 1],
                                    e_t[:, qt * P:qt * P + P],
                                    axis=mybir.AxisListType.X)
                                npart += 1

                        denom = attn.tile([P, 1], fp32, tag="denom")
                        nc.vector.reduce_sum(denom[:], dpart[:, :npart],
                                             axis=mybir.AxisListType.X)
                        recip = attn.tile([P, 1], fp32, tag="recip")
                        nc.vector.reciprocal(recip[:], denom[:])
                        nc.vector.tensor_scalar_mul(e_t[:, :cw], e_t[:, :cw],
                                                    recip[:])

                        # transpose attn blocks, then attn @ v
                        aT = attn.tile([P, L], bf16, tag="aT")
                        for kt in range(ncb):
                            if dma_transpose:
                                nc.sync.dma_start_transpose(
                                    aT[:, kt * P:(kt + 1) * P],
                                    e_t[:, kt * P:(kt + 1) * P])
                                continue
                            ps_t = ps_t_pool.tile([P, P], bf16, tag="ps_t")
                            nc.tensor.transpose(
                                ps_t[:], e_t[:, kt * P:(kt + 1) * P],
                                identity[:])
                            nc.vector.tensor_copy(aT[:, kt * P:(kt + 1) * P],
                                                  ps_t[:])
                        ps_o = ps_o_pool.tile([P, P], fp32, tag="ps_o")
                        for kt in range(ncb):
                            nc.tensor.matmul(
                                ps_o[:], lhsT=v_sb[:, kt, h * P:(h + 1) * P],
                                rhs=aT[:, kt * P:(kt + 1) * P],
                                start=(kt == 0), stop=(kt == ncb - 1))
                        nc.scalar.copy(outT_sb[:, h, qsl], ps_o[:])

                    # head h complete: AllGather its outT slice, then load
                    # the gathered rank blocks into oT (overlaps next heads)
                    nc.sync.dma_start(ag_in_h[h][:], outT_sb[:, h, :])
                    if no_cc:
                        nc.sync.dma_start(ag_out_h[h][:P, :], ag_in_h[h][:])
                    else:
                        nc.gpsimd.collective_compute(
                            "AllGather", mybir.AluOpType.bypass,
                            replica_groups=groups,
                            ins=[ag_in_h[h][:]], outs=[ag_out_h[h][:]])

            if debug_outputs:
                for h in range(NH):
                    st = persist.tile([P, L], fp32, tag="dbg_cast")
                    nc.vector.tensor_copy(st[:], outT_sb[:, h, :])
                    nc.sync.dma_start(dbg["outT"][h], st[:])

        # ---------------- stage 4: MLP ----------------
        with tc.tile_pool(name="mlp_persist", bufs=1) as mlpp, \
             tc.tile_pool(name="mlp_y", bufs=2) as mlp_y:
            hT_sb = mlpp.tile([P, EL // P, L], bf16)

            # phase A: hT = silu(oT.T W1).T * (oT.T Vg).T, column-sharded
            with tc.tile_pool(name="mlp_h", bufs=1) as mlp_h, \
                 tc.tile_pool(name="mlp_w", bufs=2) as mlp_w, \
                 tc.tile_pool(name="mlp_ps", bufs=2, space="PSUM") as mlp_ps, \
                 tc.tile_pool(name="mlp_tmp", bufs=2) as mlp_tmp:
                oT_sb = mlp_h.tile([P, DC, L], bf16)
                for h in range(NH):
                    for rr in range(TP):
                        nc.sync.dma_start(
                            oT_sb[:, rr * NH + h, :],
                            ag_out_h[h][rr * P:(rr + 1) * P, :])
                # accumulate over d in head-arrival order: chunks of heads
                # 0..2 are AllGathered before head 3 finishes, so the first
                # chains can start during the final AG.
                dc_order = [rr * NH + h for h in range(NH) for rr in range(TP)]
                for et in range(EL // P):
                    w1_sb = mlp_w.tile([P, DC, P], bf16, tag="w1")
                    nc.sync.dma_start(w1_sb[:], w1t[et])
                    vg_sb = mlp_w.tile([P, DC, P], bf16, tag="vg")
                    nc.sync.dma_start(vg_sb[:], vgt[et])
                    for pc in range(L // 512):
                        psl = slice(pc * 512, (pc + 1) * 512)
                        ps_h1 = mlp_ps.tile([P, 512], fp32, tag="ps_h1")
                        for i, dc in enumerate(dc_order):
                            nc.tensor.matmul(
                                ps_h1[:], lhsT=w1_sb[:, dc, :],
                                rhs=oT_sb[:, dc, psl],
                                start=(i == 0), stop=(i == DC - 1))
                        ps_g1 = mlp_ps.tile([P, 512], fp32, tag="ps_g1")
                        for i, dc in enumerate(dc_order):
                            nc.tensor.matmul(
                                ps_g1[:], lhsT=vg_sb[:, dc, :],
                                rhs=oT_sb[:, dc, psl],
                                start=(i == 0), stop=(i == DC - 1))
                        sil = mlp_tmp.tile([P, 512], bf16, tag="sil")
                        nc.scalar.activation(sil[:], ps_h1[:],
                                             mybir.ActivationFunctionType.Silu)
                        nc.vector.tensor_tensor(hT_sb[:, et, psl], sil[:],
                                                ps_g1[:], mybir.AluOpType.mult)

            # phase B: y = h @ W2 (row-sharded W2) -> fp32 partials,
            # ReduceScattered per 512-column chunk (overlaps compute)
            with tc.tile_pool(name="y_ps", bufs=3, space="PSUM") as y_ps, \
                 tc.tile_pool(name="y_tmp", bufs=3) as y_tmp:
                for nd in range(ND):
                    w0, cw2 = RS_OFFS[nd], RS_CHUNKS[nd]
                    w2_sb = mlp_y.tile([P, EL // P, 512], bf16, tag="w2")
                    nc.sync.dma_start(
                        w2_sb[:, :, :cw2],
                        w2[:, w0:w0 + cw2].rearrange("(c p) d -> p c d", p=P))
                    for pt in range(LT):
                        ps_y = y_ps.tile([P, 512], fp32, tag="ps_y")
                        for et in range(EL // P):
                            nc.tensor.matmul(
                                ps_y[:, :cw2],
                                lhsT=hT_sb[:, et, pt * P:(pt + 1) * P],
                                rhs=w2_sb[:, et, :cw2],
                                start=(et == 0), stop=(et == EL // P - 1))
                        yst = y_tmp.tile([P, 512], fp32, tag="yst")
                        nc.scalar.copy(yst[:, :cw2], ps_y[:, :cw2])
                        nc.sync.dma_start(
                            rs_in_n[nd][pt * P:(pt + 1) * P, :], yst[:, :cw2])
                    if no_cc:
                        nc.sync.dma_start(rs_out_n[nd][:], rs_in_n[nd][:ROWS, :])
                    else:
                        nc.gpsimd.collective_compute(
                            "ReduceScatter", mybir.AluOpType.add,
                            replica_groups=groups,
                            ins=[rs_in_n[nd][:]], outs=[rs_out_n[nd][:]])
                    nc.sync.dma_start(y_out[:, w0:w0 + cw2], rs_out_n[nd][:])

    nc.compile()
    return nc


def _prep_inputs(x, Wq, Wk, Wv, W1, Vg, W2):
    """Build the 8 per-core input maps (host-side shard + cast + tile)."""
    tri = np.tril(np.ones((P, P), np.float32)).astype(BF16)
    in_maps = []
    for core in range(NCORES):
        g, r = divmod(core, TP)
        xT = np.ascontiguousarray(x[g].T).astype(BF16)
        hsl = slice(r * HSL, (r + 1) * HSL)
        esl = slice(r * EL, (r + 1) * EL)
        w1 = W1[:, esl].astype(BF16)  # (D, EL)
        # tile to (EL//P, P, DC, P): (et, p, dc, e) = w1[dc*P+p, et*P+e]
        w1t = np.ascontiguousarray(
            w1.reshape(DC, P, EL // P, P).transpose(2, 1, 0, 3))
        vg = Vg[:, esl].astype(BF16)
        vgt = np.ascontiguousarray(
            vg.reshape(DC, P, EL // P, P).transpose(2, 1, 0, 3))
        in_maps.append({
            "xT": xT,
            "wq": np.ascontiguousarray(Wq[:, hsl]).astype(BF16),
            "wk": np.ascontiguousarray(Wk[:, hsl]).astype(BF16),
            "wv": np.ascontiguousarray(Wv[:, hsl]).astype(BF16),
            "w1t": w1t,
            "vgt": vgt,
            "w2": np.ascontiguousarray(W2[esl, :]).astype(BF16),
            "tri": tri,
        })
    return in_maps


def run(inputs, trace=False, debug_outputs=False):
    """Compile (cached) and run; returns (y, BassKernelResults)."""
    global _PROGRAM
    from concourse import bass_utils

    if debug_outputs:
        nc = _build_program(debug_outputs=True)
    else:
        if _PROGRAM is None:
            _PROGRAM = _build_program()
        nc = _PROGRAM

    in_maps = _prep_inputs(inputs["x"], inputs["Wq"], inputs["Wk"],
                           inputs["Wv"], inputs["W1"], inputs["Vg"],
                           inputs["W2"])
    res = bass_utils.run_bass_kernel_spmd(
        nc, in_maps, core_ids=list(range(NCORES)), trace=trace)
    y = np.empty((B, L, D), np.float32)
    for core in range(NCORES):
        g, r = divmod(core, TP)
        y[g, r * ROWS:(r + 1) * ROWS, :] = res.results[core]["y"]
    return y, res


def kernel(**inputs):
    y, _ = run(inputs)
    return y


# revision 22
# speedup vs baseline: 1.0380x; 1.0278x over previous
"""Trainium2 Bass kernel for nn_ExpertAttentionHead (attention + SwiGLU MLP).

Sharding (8 cores): DP over batch (2 groups of 4 cores) x TP within group.
  - Attention: heads split 4-way (4 heads/core). QKV projections computed in
    transposed layout (hd on partitions) from host-pretransposed x^T.
  - Per-head AllGather of the attention output (transposed layout) within
    each group, overlapped with attention of the remaining heads.
  - MLP: W1/Vg column-sharded (E/4 per core), W2 row-sharded; the fp32
    partial outputs are ReduceScattered in 4 column chunks overlapped with
    compute; the host concatenates row slices.

Everything is hardcoded for B=2, L=2048, D=2048, H=16, HD=128, E=8192.
"""

import sys

import numpy as np

sys.path.insert(0, "/opt/trn_rl_repo")

import ml_dtypes

BF16 = ml_dtypes.bfloat16

B, L, D = 2, 2048, 2048
H, HD = 16, 128
E = 8192
SCALE = float(np.sqrt(HD))

P = 128
NCORES = 8
TP = 4  # tensor-parallel ranks per group
NH = H // TP  # local heads = 4
HSL = NH * HD  # head col slice width = 512
EL = E // TP  # local E = 2048
LT = L // P  # 16 query tiles
DC = D // P  # 16 contraction chunks
ROWS = L // TP  # 512 output rows per core
RS_CHUNKS = [512, 512, 512, 256, 256]  # ReduceScatter column chunks
RS_OFFS = [0, 512, 1024, 1536, 1792]
ND = len(RS_CHUNKS)

_PROGRAM = None


def _build_program(debug_outputs=False, no_cc=False, dma_transpose=False):
    import concourse.bacc as bacc
    import concourse.mybir as mybir
    import concourse.tile as tile
    from concourse.masks import make_identity

    fp32 = mybir.dt.float32
    bf16 = mybir.dt.bfloat16

    nc = bacc.Bacc("TRN2", target_bir_lowering=False, debug=False,
                   num_devices=NCORES)

    # ---- I/O ----
    xT = nc.dram_tensor("xT", [D, L], bf16, kind="ExternalInput")
    wq = nc.dram_tensor("wq", [D, HSL], bf16, kind="ExternalInput")
    wk = nc.dram_tensor("wk", [D, HSL], bf16, kind="ExternalInput")
    wv = nc.dram_tensor("wv", [D, HSL], bf16, kind="ExternalInput")
    # host-tiled: (E-tile, p=d_in_chunk, d-chunk, e_cols)
    w1t = nc.dram_tensor("w1t", [EL // P, P, DC, P], bf16, kind="ExternalInput")
    vgt = nc.dram_tensor("vgt", [EL // P, P, DC, P], bf16, kind="ExternalInput")
    w2 = nc.dram_tensor("w2", [EL, D], bf16, kind="ExternalInput")
    tri = nc.dram_tensor("tri", [P, P], bf16, kind="ExternalInput")

    y_out = nc.dram_tensor("y", [ROWS, D], fp32, kind="ExternalOutput")

    # ---- collective bounce buffers (internal DRAM) ----
    ag_in_h = [nc.dram_tensor(f"ag_in_{h}", [P, L], bf16) for h in range(NH)]
    ag_out_h = [nc.dram_tensor(f"ag_out_{h}", [TP * P, L], bf16)
                for h in range(NH)]
    rs_in_n = [nc.dram_tensor(f"rs_in_{n}", [L, RS_CHUNKS[n]], fp32)
               for n in range(ND)]
    rs_out_n = [nc.dram_tensor(f"rs_out_{n}", [ROWS, RS_CHUNKS[n]], fp32)
                for n in range(ND)]

    groups = [[0, 1, 2, 3], [4, 5, 6, 7]]

    dbg = {}
    if debug_outputs:
        dbg["qT"] = nc.dram_tensor("dbg_qT", [NH, P, L], fp32, kind="ExternalOutput")
        dbg["kT"] = nc.dram_tensor("dbg_kT", [NH, P, L], fp32, kind="ExternalOutput")
        dbg["v"] = nc.dram_tensor("dbg_v", [LT, P, HSL], fp32, kind="ExternalOutput")
        dbg["outT"] = nc.dram_tensor("dbg_outT", [NH, P, L], fp32,
                                     kind="ExternalOutput")

    with tile.TileContext(nc) as tc, \
         tc.tile_pool(name="consts", bufs=1) as consts:
        identity = consts.tile([P, P], bf16)
        make_identity(nc, identity[:])
        tri_sb = consts.tile([P, P], bf16)
        nc.sync.dma_start(tri_sb[:], tri[:])

        # persistent across stage 1+2
        with tc.tile_pool(name="attn_persist", bufs=1) as persist:
            qT_sb = persist.tile([P, NH, L], bf16)
            kT_sb = persist.tile([P, NH, L], bf16)
            v_sb = persist.tile([P, LT, HSL], bf16)
            kbar_sb = persist.tile([P, NH], bf16)
            outT_sb = persist.tile([P, NH, L], bf16)

            # ---------------- stage 1: QKV projections ----------------
            with tc.tile_pool(name="proj", bufs=1) as proj, \
                 tc.tile_pool(name="proj_ps", bufs=3, space="PSUM") as proj_ps:
                wq_sb = proj.tile([P, DC, HSL], bf16, tag="wq")
                wq_r = wq.rearrange("(c p) h -> p c h", p=P)
                xT_sb = proj.tile([P, DC, L], bf16, tag="xT")
                xT_r = xT.rearrange("(c p) l -> p c l", p=P)
                wk_sb = proj.tile([P, DC, HSL], bf16, tag="wk")
                wk_r = wk.rearrange("(c p) h -> p c h", p=P)
                wv_sb = proj.tile([P, DC, HSL], bf16, tag="wv")
                # DMA emission order == consumption order: head-0 weight
                # d-chunks interleaved with the matching x^T d-subchunks of
                # the first pos-quarter, then the rest in consumption order.
                for s in range(4):
                    sl = slice(4 * s, 4 * (s + 1))
                    nc.sync.dma_start(wq_sb[:, sl, 0:P], wq_r[:, sl, 0:P])
                    nc.sync.dma_start(xT_sb[:, sl, 0:512], xT_r[:, sl, 0:512])
                for h in range(1, NH):
                    nc.sync.dma_start(wq_sb[:, :, h * P:(h + 1) * P],
                                      wq_r[:, :, h * P:(h + 1) * P])
                for j in range(1, 4):
                    nc.sync.dma_start(xT_sb[:, :, j * 512:(j + 1) * 512],
                                      xT_r[:, :, j * 512:(j + 1) * 512])
                for h in range(NH):
                    nc.sync.dma_start(wk_sb[:, :, h * P:(h + 1) * P],
                                      wk_r[:, :, h * P:(h + 1) * P])
                nc.sync.dma_start(wv_sb[:], wv.rearrange("(c p) h -> p c h", p=P))

                # q^T, k^T per head: (hd=128, pos) = sum_d W[:,h]^T x^T
                # pc-outer so the first chains only need the first x^T quarter
                for w_sb, dst in ((wq_sb, qT_sb), (wk_sb, kT_sb)):
                    for pc in range(L // 512):
                        for h in range(NH):
                            ps = proj_ps.tile([P, 512], fp32, tag="proj_ps")
                            for dc in range(DC):
                                nc.tensor.matmul(
                                    ps[:],
                                    lhsT=w_sb[:, dc, h * P:(h + 1) * P],
                                    rhs=xT_sb[:, dc, pc * 512:(pc + 1) * 512],
                                    start=(dc == 0), stop=(dc == DC - 1),
                                )
                            nc.scalar.copy(dst[:, h, pc * 512:(pc + 1) * 512], ps[:])
                # v in normal layout: (pos, hd-cols)
                for pt in range(LT):
                    ps = proj_ps.tile([P, HSL], fp32, tag="proj_ps")
                    for dc in range(DC):
                        nc.tensor.matmul(
                            ps[:],
                            lhsT=xT_sb[:, dc, pt * P:(pt + 1) * P],
                            rhs=wv_sb[:, dc, :],
                            start=(dc == 0), stop=(dc == DC - 1),
                        )
                    nc.scalar.copy(v_sb[:, pt, :], ps[:])

            # k_bar per head (sum over keys) for the Reynolds row-mean
            for h in range(NH):
                kbar_f = persist.tile([P, 1], fp32, tag="kbar_f")
                nc.vector.reduce_sum(kbar_f[:], kT_sb[:, h, :],
                                     axis=mybir.AxisListType.X)
                nc.vector.tensor_copy(kbar_sb[:, h:h + 1], kbar_f[:])

            if debug_outputs:
                for h in range(NH):
                    st = persist.tile([P, L], fp32, tag="dbg_cast")
                    nc.vector.tensor_copy(st[:], qT_sb[:, h, :])
                    nc.sync.dma_start(dbg["qT"][h], st[:])
                for h in range(NH):
                    st = persist.tile([P, L], fp32, tag="dbg_cast")
                    nc.vector.tensor_copy(st[:], kT_sb[:, h, :])
                    nc.sync.dma_start(dbg["kT"][h], st[:])
                for pt in range(LT):
                    st = persist.tile([P, HSL], fp32, tag="dbg_cast2")
                    nc.vector.tensor_copy(st[:], v_sb[:, pt, :])
                    nc.sync.dma_start(dbg["v"][pt], st[:])

            # ---------------- stage 2: attention + per-head AllGather ------
            with tc.tile_pool(name="attn", bufs=3) as attn, \
                 tc.tile_pool(name="ps_s", bufs=3, space="PSUM") as ps_s_pool, \
                 tc.tile_pool(name="ps_r", bufs=1, space="PSUM") as ps_r_pool, \
                 tc.tile_pool(name="ps_t", bufs=2, space="PSUM") as ps_t_pool, \
                 tc.tile_pool(name="ps_o", bufs=2, space="PSUM") as ps_o_pool:
                for h in range(NH):
                    for qt in range(LT):
                        ncb = qt + 1          # causal key blocks
                        cw = ncb * P          # causal width
                        qsl = slice(qt * P, (qt + 1) * P)

                        # Reynolds row-mean via k_bar: rowsum = q . k_bar
                        ps_row = ps_r_pool.tile([P, 1], fp32, tag="ps_row")
                        nc.tensor.matmul(ps_row[:], lhsT=qT_sb[:, h, qsl],
                                         rhs=kbar_sb[:, h:h + 1],
                                         start=True, stop=True)
                        bias_t = attn.tile([P, 1], fp32, tag="bias")
                        nc.vector.tensor_scalar_mul(
                            bias_t[:], ps_row[:], 0.5 / (SCALE * L))

                        e_t = attn.tile([P, L], bf16, tag="e")
                        dpart = attn.tile([P, 8], fp32, tag="dpart")
                        npart = 0
                        # causal chunks of <=512 keys
                        for c0 in range(0, cw, 512):
                            w = min(512, cw - c0)
                            ps_sc = ps_s_pool.tile([P, 512], fp32, tag="ps_s")
                            nc.tensor.matmul(
                                ps_sc[:, :w], lhsT=qT_sb[:, h, qsl],
                                rhs=kT_sb[:, h, c0:c0 + w],
                                start=True, stop=True)
                            # exp(0.5*s/SCALE + bias), accumulate row-sums
                            pre_w = w if c0 + w <= qt * P else w - P
                            if pre_w > 0:
                                nc.scalar.activation(
                                    e_t[:, c0:c0 + pre_w], ps_sc[:, :pre_w],
                                    mybir.ActivationFunctionType.Exp,
                                    bias=bias_t[:], scale=0.5 / SCALE,
                                    accum_out=dpart[:, npart:npart + 1])
                                npart += 1
                            if c0 + w > qt * P:
                                # diagonal block: exp, tri-mask, row-sum
                                doff = qt * P - c0
                                nc.scalar.activation(
                                    e_t[:, qt * P:qt * P + P],
                                    ps_sc[:, doff:doff + P],
                                    mybir.ActivationFunctionType.Exp,
                                    bias=bias_t[:], scale=0.5 / SCALE)
                                nc.vector.tensor_tensor(
                                    e_t[:, qt * P:qt * P + P],
                                    e_t[:, qt * P:qt * P + P], tri_sb[:],
                                    mybir.AluOpType.mult)
                                nc.vector.reduce_sum(
                                    dpart[:, npart:npart + 1],
                                    e_t[:, qt * P:qt * P + P],
                                    axis=mybir.AxisListType.X)
                                npart += 1

                        denom = attn.tile([P, 1], fp32, tag="denom")
                        nc.vector.reduce_sum(denom[:], dpart[:, :npart],
                                             axis=mybir.AxisListType.X)
                        recip = attn.tile([P, 1], fp32, tag="recip")
                        nc.vector.reciprocal(recip[:], denom[:])
                        nc.vector.tensor_scalar_mul(e_t[:, :cw], e_t[:, :cw],
                                                    recip[:])

                        # transpose attn blocks, then attn @ v
                        aT = attn.tile([P, L], bf16, tag="aT")
                        for kt in range(ncb):
                            if dma_transpose:
                                nc.sync.dma_start_transpose(
                                    aT[:, kt * P:(kt + 1) * P],
                                    e_t[:, kt * P:(kt + 1) * P])
                                continue
                            ps_t = ps_t_pool.tile([P, P], bf16, tag="ps_t")
                            nc.tensor.transpose(
                                ps_t[:], e_t[:, kt * P:(kt + 1) * P],
                                identity[:])
                            nc.vector.tensor_copy(aT[:, kt * P:(kt + 1) * P],
                                                  ps_t[:])
                        ps_o = ps_o_pool.tile([P, P], fp32, tag="ps_o")
                        for kt in range(ncb):
                            nc.tensor.matmul(
                                ps_o[:], lhsT=v_sb[:, kt, h * P:(h + 1) * P],
                                rhs=aT[:, kt * P:(kt + 1) * P],
                                start=(kt == 0), stop=(kt == ncb - 1))
                        nc.scalar.copy(outT_sb[:, h, qsl], ps_o[:])

                    # head h complete: AllGather its outT slice, then load
                    # the gathered rank blocks into oT (overlaps next heads)
                    nc.sync.dma_start(ag_in_h[h][:], outT_sb[:, h, :])
                    if no_cc:
                        nc.sync.dma_start(ag_out_h[h][:P, :], ag_in_h[h][:])
                    else:
                        nc.gpsimd.collective_compute(
                            "AllGather", mybir.AluOpType.bypass,
                            replica_groups=groups,
                            ins=[ag_in_h[h][:]], outs=[ag_out_h[h][:]])

            if debug_outputs:
                for h in range(NH):
                    st = persist.tile([P, L], fp32, tag="dbg_cast")
                    nc.vector.tensor_copy(st[:], outT_sb[:, h, :])
                    nc.sync.dma_start(dbg["outT"][h], st[:])

        # ---------------- stage 4: MLP ----------------
        with tc.tile_pool(name="mlp_persist", bufs=1) as mlpp, \
             tc.tile_pool(name="mlp_y", bufs=2) as mlp_y:
            hT_sb = mlpp.tile([P, EL // P, L], bf16)

            # phase A: hT = silu(oT.T W1).T * (oT.T Vg).T, column-sharded
            with tc.tile_pool(name="mlp_h", bufs=1) as mlp_h, \
                 tc.tile_pool(name="mlp_w", bufs=2) as mlp_w, \
                 tc.tile_pool(name="mlp_ps", bufs=4, space="PSUM") as mlp_ps, \
                 tc.tile_pool(name="mlp_tmp", bufs=2) as mlp_tmp:
                oT_sb = mlp_h.tile([P, DC, L], bf16)
                for h in range(NH):
                    for rr in range(TP):
                        nc.sync.dma_start(
                            oT_sb[:, rr * NH + h, :],
                            ag_out_h[h][rr * P:(rr + 1) * P, :])
                # accumulate over d in head-arrival order: chunks of heads
                # 0..2 are AllGathered before head 3 finishes, so the first
                # chains can start during the final AG.
                dc_order = [rr * NH + h for h in range(NH) for rr in range(TP)]
                for et in range(EL // P):
                    w1_sb = mlp_w.tile([P, DC, P], bf16, tag="w1")
                    nc.sync.dma_start(w1_sb[:], w1t[et])
                    vg_sb = mlp_w.tile([P, DC, P], bf16, tag="vg")
                    nc.sync.dma_start(vg_sb[:], vgt[et])
                    for pc in range(L // 512):
                        psl = slice(pc * 512, (pc + 1) * 512)
                        ps_h1 = mlp_ps.tile([P, 512], fp32, tag="ps_h1")
                        for i, dc in enumerate(dc_order):
                            nc.tensor.matmul(
                                ps_h1[:], lhsT=w1_sb[:, dc, :],
                                rhs=oT_sb[:, dc, psl],
                                start=(i == 0), stop=(i == DC - 1))
                        ps_g1 = mlp_ps.tile([P, 512], fp32, tag="ps_g1")
                        for i, dc in enumerate(dc_order):
                            nc.tensor.matmul(
                                ps_g1[:], lhsT=vg_sb[:, dc, :],
                                rhs=oT_sb[:, dc, psl],
                                start=(i == 0), stop=(i == DC - 1))
                        sil = mlp_tmp.tile([P, 512], bf16, tag="sil")
                        nc.scalar.activation(sil[:], ps_h1[:],
                                             mybir.ActivationFunctionType.Silu)
                        nc.vector.tensor_tensor(hT_sb[:, et, psl], sil[:],
                                                ps_g1[:], mybir.AluOpType.mult)

            # phase B: y = h @ W2 (row-sharded W2) -> fp32 partials,
            # ReduceScattered per 512-column chunk (overlaps compute)
            with tc.tile_pool(name="y_ps", bufs=4, space="PSUM") as y_ps, \
                 tc.tile_pool(name="y_tmp", bufs=3) as y_tmp:
                for nd in range(ND):
                    w0, cw2 = RS_OFFS[nd], RS_CHUNKS[nd]
                    w2_sb = mlp_y.tile([P, EL // P, 512], bf16, tag="w2")
                    nc.sync.dma_start(
                        w2_sb[:, :, :cw2],
                        w2[:, w0:w0 + cw2].rearrange("(c p) d -> p c d", p=P))
                    for pt in range(LT):
                        ps_y = y_ps.tile([P, 512], fp32, tag="ps_y")
                        for et in range(EL // P):
                            nc.tensor.matmul(
                                ps_y[:, :cw2],
                                lhsT=hT_sb[:, et, pt * P:(pt + 1) * P],
                                rhs=w2_sb[:, et, :cw2],
                                start=(et == 0), stop=(et == EL // P - 1))
                        yst = y_tmp.tile([P, 512], fp32, tag="yst")
                        nc.scalar.copy(yst[:, :cw2], ps_y[:, :cw2])
                        nc.sync.dma_start(
                            rs_in_n[nd][pt * P:(pt + 1) * P, :], yst[:, :cw2])
                    if no_cc:
                        nc.sync.dma_start(rs_out_n[nd][:], rs_in_n[nd][:ROWS, :])
                    else:
                        nc.gpsimd.collective_compute(
                            "ReduceScatter", mybir.AluOpType.add,
                            replica_groups=groups,
                            ins=[rs_in_n[nd][:]], outs=[rs_out_n[nd][:]])
                    nc.sync.dma_start(y_out[:, w0:w0 + cw2], rs_out_n[nd][:])

    nc.compile()
    return nc


def _prep_inputs(x, Wq, Wk, Wv, W1, Vg, W2):
    """Build the 8 per-core input maps (host-side shard + cast + tile)."""
    tri = np.tril(np.ones((P, P), np.float32)).astype(BF16)
    in_maps = []
    for core in range(NCORES):
        g, r = divmod(core, TP)
        xT = np.ascontiguousarray(x[g].T).astype(BF16)
        hsl = slice(r * HSL, (r + 1) * HSL)
        esl = slice(r * EL, (r + 1) * EL)
        w1 = W1[:, esl].astype(BF16)  # (D, EL)
        # tile to (EL//P, P, DC, P): (et, p, dc, e) = w1[dc*P+p, et*P+e]
        w1t = np.ascontiguousarray(
            w1.reshape(DC, P, EL // P, P).transpose(2, 1, 0, 3))
        vg = Vg[:, esl].astype(BF16)
        vgt = np.ascontiguousarray(
            vg.reshape(DC, P, EL // P, P).transpose(2, 1, 0, 3))
        in_maps.append({
            "xT": xT,
            "wq": np.ascontiguousarray(Wq[:, hsl]).astype(BF16),
            "wk": np.ascontiguousarray(Wk[:, hsl]).astype(BF16),
            "wv": np.ascontiguousarray(Wv[:, hsl]).astype(BF16),
            "w1t": w1t,
            "vgt": vgt,
            "w2": np.ascontiguousarray(W2[esl, :]).astype(BF16),
            "tri": tri,
        })
    return in_maps


def run(inputs, trace=False, debug_outputs=False):
    """Compile (cached) and run; returns (y, BassKernelResults)."""
    global _PROGRAM
    from concourse import bass_utils

    if debug_outputs:
        nc = _build_program(debug_outputs=True)
    else:
        if _PROGRAM is None:
            _PROGRAM = _build_program()
        nc = _PROGRAM

    in_maps = _prep_inputs(inputs["x"], inputs["Wq"], inputs["Wk"],
                           inputs["Wv"], inputs["W1"], inputs["Vg"],
                           inputs["W2"])
    res = bass_utils.run_bass_kernel_spmd(
        nc, in_maps, core_ids=list(range(NCORES)), trace=trace)
    y = np.empty((B, L, D), np.float32)
    for core in range(NCORES):
        g, r = divmod(core, TP)
        y[g, r * ROWS:(r + 1) * ROWS, :] = res.results[core]["y"]
    return y, res


def kernel(**inputs):
    y, _ = run(inputs)
    return y
